# revision 14
# baseline (speedup 1.0000x reference)
"""BFP-quantized linear (nn_BFPLinear) on 8 Trainium2 NeuronCores.

Math (must match reference exactly):
    xq = bfp_quant8_g64(x); wq = bfp_quant8_g64(weight)
    out = xq @ wq.T + 2*bias

Sharding (2 row-groups x 4 col-groups, core c = 4r+k), collective-free:
  - core c loads x rows [2048r, 2048(r+1)) and weight rows
    [1024k, 1024(k+1)) as fp32 and quantizes them on-chip (x is
    quantized redundantly by the 4 cores of each row group, w by the
    2 cores sharing a col shard; this removes all inter-core traffic).
  - output shard per core: out[2048r:2048(r+1), 1024k:1024(k+1)],
    written contiguously (no host-side permutation).

Quantization per group of 64 along `in` (bit-exact vs the fp32 ref):
    gmax = max|x|; p2 = exponent-only bits of gmax (= 2^e), clamped to
    >= 2^-126; inv = bitcast(0x7F000000 - p2) = 2^-e (exact);
    y = x * inv (exact power-of-2 scale);
    y = clip(y, -128.49/128, 127.49/128);
    m = (y + 1.5*2^16) - 1.5*2^16   (fused DVE round-to-nearest-even at
        ulp 2^-7, verified bit-exact on HW), cast to bf16 (exact);
    xq = m * bf16(p2) = round(x/step)*step with step = 2^(e-7), exact
        in bf16 (|m| <= 1, 8-bit mantissa, power-of-2 step).
The bf16 matmul inputs equal the reference's fp32 quantized values
bit-for-bit, so the only output error is fp32 summation order.

Transposes (quantized bf16 -> contraction-on-partition layout) are done
with SBUF->SBUF xbar DMA (dma_start_transpose), no DRAM staging.

build(loop_reps=N) wraps the whole body in a hardware For_i loop: one
NEFF executes the kernel N times back-to-back with no host round trip.
This is used by time_kernel() to measure per-execution device time far
below the host dispatch jitter. The graded kernel() path uses the plain
(no-loop) build.
"""
import sys

sys.path.insert(0, "/opt/trn_rl_repo")

import numpy as np

import concourse.bass as bass
import concourse.tile as tile
from concourse import mybir, bacc
from concourse.bass_utils import run_bass_kernel_spmd

# problem shape (hardcoded; kernel.py must be self-contained)
N = 4096
IN = 4096
OUT = 4096
NCORES = 8
RGRP = 2                    # row groups (x sharded 2-way)
CGRP = 4                    # col groups (weight sharded 4-way)
NLOC = N // RGRP            # 2048 output rows per core
WSH = OUT // CGRP           # 1024 output cols per core
J = 64                      # bfp group size
G = IN // J                 # 64 groups per row
KT = IN // 128              # 32 contraction tiles
P = 128
NXT = NLOC // P             # 16 x row-tiles per core
NWT = WSH // P              # 8 w row-tiles per core

MASK_EXP = 0x7F800000
MIN_NORM = 0x00800000
INV_C = 0x7F000000
MAGIC = float(np.float32(1.5 * 2.0 ** 16))
CLIP_HI = float(np.float32(127.49 / 128.0))
CLIP_LO = float(np.float32(-128.49 / 128.0))

_CACHE = {}


def build(loop_reps=None, ablate=None):
    """loop_reps=None: plain one-shot body (graded path).
    loop_reps=k: body wrapped in a hardware For_i executing k times.
    ablate: None | 'empty' | 'mmonly' | 'qtonly' | 'qonly' (perf probes)."""
    dt = mybir.dt
    nc = bacc.Bacc("TRN2", target_bir_lowering=False, debug=False)
    x_d = nc.dram_tensor("x_own", [NLOC, IN], dt.float32,
                         kind="ExternalInput").ap()
    w_d = nc.dram_tensor("w_own", [WSH, IN], dt.float32,
                         kind="ExternalInput").ap()
    b_d = nc.dram_tensor("bias2_rep", [P, WSH], dt.float32,
                         kind="ExternalInput").ap()
    out_d = nc.dram_tensor("out", [NLOC, WSH], dt.float32,
                           kind="ExternalOutput").ap()

    with tile.TileContext(nc) as tc:
        with tc.tile_pool(name="sb", bufs=1) as sb, \
             tc.tile_pool(name="inp", bufs=2) as inp, \
             tc.tile_pool(name="ypool", bufs=2) as ypool, \
             tc.tile_pool(name="mpool", bufs=3) as mpool, \
             tc.tile_pool(name="xqt", bufs=3) as xqtp, \
             tc.tile_pool(name="small", bufs=4) as small, \
             tc.tile_pool(name="otp", bufs=2) as otp, \
             tc.tile_pool(name="psum", bufs=4, space="PSUM") as psump:

            # constants (outside the timing loop)
            bias2 = sb.tile([P, WSH], dt.float32)
            nc.sync.dma_start(bias2[:], b_d)
            magic_p = sb.tile([P, 1], dt.float32)
            nc.vector.memset(magic_p[:], MAGIC)
            magic_n = sb.tile([P, 1], dt.float32)
            nc.vector.memset(magic_n[:], -MAGIC)
            # wqT[p, kt, o]: contraction i = kt*128+p, o = out col
            wqT = sb.tile([P, KT, WSH], dt.bfloat16)

            def quantize(src_d, row):
                """Load fp32 rows [row*128,(row+1)*128) of src_d, return
                quantized bf16 tile [128, IN] (= m*step, bit-exact)."""
                xt = inp.tile([P, IN], dt.float32, tag="in", name="in")
                nc.scalar.dma_start(xt[:], src_d[row * P:(row + 1) * P, :])
                x3 = xt[:].rearrange("p (g j) -> p g j", j=J)
                gmax = small.tile([P, G], dt.float32, tag="gmax", name="gmax")
                nc.vector.tensor_reduce(gmax[:], x3, mybir.AxisListType.X,
                                        mybir.AluOpType.max,
                                        apply_absolute_value=True)
                p2 = small.tile([P, G], dt.int32, tag="p2", name="p2")
                nc.vector.tensor_scalar(p2[:], gmax[:].bitcast(dt.int32),
                                        MASK_EXP, None,
                                        mybir.AluOpType.bitwise_and)
                nc.vector.tensor_scalar(p2[:], p2[:], MIN_NORM, None,
                                        mybir.AluOpType.max)
                inv_i = small.tile([P, G], dt.int32, tag="invi", name="invi")
                nc.vector.tensor_scalar(inv_i[:], p2[:], -1, INV_C,
                                        mybir.AluOpType.mult,
                                        mybir.AluOpType.add)
                step_bf = small.tile([P, G], dt.bfloat16, tag="stepbf",
                                     name="stepbf")
                nc.vector.tensor_copy(step_bf[:], p2[:].bitcast(dt.float32))

                yt = ypool.tile([P, IN], dt.float32, tag="y", name="y")
                y3 = yt[:].rearrange("p (g j) -> p g j", j=J)
                inv_b = (inv_i[:].bitcast(dt.float32).unsqueeze(2)
                         .broadcast_to([P, G, J]))
                # normalize mult split DVE/GpSimd (measured: DVE 4.6us,
                # GpSimd 10.7us per full tile -> 20/44 group split)
                GM = 20
                nc.vector.tensor_tensor(y3[:, 0:GM, :], x3[:, 0:GM, :],
                                        inv_b[:, 0:GM, :],
                                        mybir.AluOpType.mult)
                nc.gpsimd.tensor_tensor(y3[:, GM:G, :], x3[:, GM:G, :],
                                        inv_b[:, GM:G, :],
                                        mybir.AluOpType.mult)
                # round first (no pre-clip): |y| < 2 so m = rne(y*128)/128
                # stays in [-2, 2], exact in bf16; then saturate in bf16
                # -- equivalent to clip-before-round.
                nc.scalar.activation(yt[:], yt[:],
                                     mybir.ActivationFunctionType.Identity,
                                     bias=magic_p[:])
                mt = mpool.tile([P, IN], dt.bfloat16, tag="m", name="m")
                nc.scalar.activation(mt[:], yt[:],
                                     mybir.ActivationFunctionType.Identity,
                                     bias=magic_n[:])
                nc.gpsimd.tensor_scalar(mt[:], mt[:], 127.0 / 128.0, -1.0,
                                        mybir.AluOpType.min,
                                        mybir.AluOpType.max)
                m3 = mt[:].rearrange("p (g j) -> p g j", j=J)
                step_b = step_bf[:].unsqueeze(2).broadcast_to([P, G, J])
                nc.vector.tensor_tensor(m3, m3, step_b, mybir.AluOpType.mult)
                return mt

            def do_w_tile(jw):
                wq = quantize(w_d, jw)
                nc.sync.dma_start_transpose(
                    wqT[:, :, jw * P:(jw + 1) * P], wq[:])

            def do_x_quant(jn):
                xq = quantize(x_d, jn)
                xqT = xqtp.tile([P, KT, P], dt.bfloat16, tag="xqT",
                                name="xqT")
                nc.sync.dma_start_transpose(xqT[:], xq[:])
                return xqT

            def do_x_mm(jn, xqT):
                ot = otp.tile([P, WSH], dt.float32, tag="ot", name="ot")
                for oh in range(2):
                    ps = psump.tile([P, 512], dt.float32, tag="ps", name="ps")
                    for kt in range(KT):
                        nc.tensor.matmul(
                            ps[:], xqT[:, kt, :],
                            wqT[:, kt, oh * 512:(oh + 1) * 512],
                            start=(kt == 0), stop=(kt == KT - 1))
                    nc.vector.tensor_tensor(
                        ot[:, oh * 512:(oh + 1) * 512], ps[:],
                        bias2[:, oh * 512:(oh + 1) * 512],
                        mybir.AluOpType.add)
                nc.scalar.dma_start(out_d[jn * P:(jn + 1) * P, :], ot[:])

            def body():
                # interleave w tiles with early x quantize/transpose so
                # all engines fill from the start. Matmuls are emitted
                # only after ALL w transposes: wqT is written in column
                # slices, and a matmul emitted earlier would read slices
                # not yet recorded as written (-> garbage).
                order = [("w", 0), ("w", 1), ("x", 0), ("w", 2), ("w", 3),
                         ("x", 1), ("w", 4), ("w", 5), ("x", 2),
                         ("w", 6), ("w", 7)]
                pending = {}
                for kind, idx in order:
                    if kind == "w":
                        do_w_tile(idx)
                    else:
                        pending[idx] = do_x_quant(idx)
                for jn in sorted(pending):
                    do_x_mm(jn, pending[jn])
                for jn in range(3, NXT):
                    xqT = do_x_quant(jn)
                    do_x_mm(jn, xqT)

            def body_empty():
                junk = otp.tile([P, 8], dt.float32, tag="ot", name="ot")
                nc.vector.memset(junk[:], 1.0)

            def body_mmonly(static_xqT):
                for jn in range(NXT):
                    do_x_mm(jn, static_xqT[jn % len(static_xqT)])

            def body_qt(with_transpose):
                for jw in range(NWT):
                    if with_transpose:
                        do_w_tile(jw)
                    else:
                        wq = quantize(w_d, jw)
                        nc.scalar.dma_start(
                            out_d[0:P, 64 * jw:64 * (jw + 1)],
                            wq[:, 0:128].bitcast(dt.float32))
                for jn in range(NXT):
                    if with_transpose:
                        xqT = do_x_quant(jn)
                        # tiny consumer so DCE keeps the chain
                        nc.scalar.dma_start(
                            out_d[jn * P:(jn + 1) * P, 0:64],
                            xqT[:, 0, :].bitcast(dt.float32))
                    else:
                        xq = quantize(x_d, jn)
                        nc.scalar.dma_start(
                            out_d[jn * P:(jn + 1) * P, 0:64],
                            xq[:, 0:128].bitcast(dt.float32))

            def body_op(op):
                xt = inp.tile([P, IN], dt.float32, tag="in", name="in")
                nc.scalar.dma_start(xt[:], x_d[0:P, :])
                x3 = xt[:].rearrange("p (g j) -> p g j", j=J)
                sc = small.tile([P, G], dt.float32, tag="gmax", name="gmax")
                nc.vector.memset(sc[:], 1.0)
                sc_b = sc[:].unsqueeze(2).broadcast_to([P, G, J])
                yt = ypool.tile([P, IN], dt.float32, tag="y", name="y")
                y3 = yt[:].rearrange("p (g j) -> p g j", j=J)
                mt = mpool.tile([P, IN], dt.bfloat16, tag="m", name="m")
                nc.vector.memset(mt[:], 0.5)
                m3 = mt[:].rearrange("p (g j) -> p g j", j=J)
                sbf = small.tile([P, G], dt.bfloat16, tag="stepbf",
                                 name="stepbf")
                nc.vector.memset(sbf[:], 1.0)
                sbf_b = sbf[:].unsqueeze(2).broadcast_to([P, G, J])
                for i in range(24):
                    if op == "gmult":
                        nc.gpsimd.tensor_tensor(y3, x3, sc_b,
                                                mybir.AluOpType.mult)
                    elif op == "dmult":
                        nc.vector.tensor_tensor(y3, x3, sc_b,
                                                mybir.AluOpType.mult)
                    elif op == "reduce":
                        nc.vector.tensor_reduce(sc[:], x3,
                                                mybir.AxisListType.X,
                                                mybir.AluOpType.max,
                                                apply_absolute_value=True)
                    elif op == "act":
                        nc.scalar.activation(
                            yt[:], xt[:],
                            mybir.ActivationFunctionType.Identity,
                            bias=magic_p[:])
                    elif op == "clip":
                        nc.vector.tensor_scalar(mt[:], mt[:], 127.0 / 128.0,
                                                -1.0, mybir.AluOpType.min,
                                                mybir.AluOpType.max)
                    elif op == "final":
                        nc.vector.tensor_tensor(m3, m3, sbf_b,
                                                mybir.AluOpType.mult)
                    elif op == "gfinal":
                        nc.gpsimd.tensor_tensor(m3, m3, sbf_b,
                                                mybir.AluOpType.mult)
                junk = otp.tile([P, 8], dt.float32, tag="ot", name="ot")
                nc.vector.tensor_copy(junk[:], yt[:, 0:8])
                nc.vector.tensor_copy(junk[:].bitcast(dt.bfloat16)[:, 0:8],
                                      mt[:, 0:8])
                nc.scalar.dma_start(out_d[0:P, 0:8], junk[:])

            if ablate is not None and ablate.startswith("op:"):
                opname = ablate[3:]
                chosen = lambda: body_op(opname)
            elif ablate == "mmonly":
                static_xqT = []
                for i in range(3):
                    t = xqtp.tile([P, KT, P], dt.bfloat16, tag="xqT",
                                  name="xqT")
                    nc.vector.memset(t[:].rearrange("p a b -> p (a b)"), 0.25)
                    static_xqT.append(t)
                nc.vector.memset(wqT[:].rearrange("p a b -> p (a b)"), 0.25)
                chosen = lambda: body_mmonly(static_xqT)
            elif ablate == "empty":
                chosen = body_empty
            elif ablate == "qtonly":
                chosen = lambda: body_qt(True)
            elif ablate == "qonly":
                chosen = lambda: body_qt(False)
            else:
                chosen = body

            if loop_reps is None:
                chosen()
            else:
                with tc.For_i(0, loop_reps, 1):
                    chosen()

    nc.compile()
    return nc


def _get_nc():
    if "nc" not in _CACHE:
        _CACHE["nc"] = build()
    return _CACHE["nc"]


def _in_maps(x, weight, bias):
    maps = []
    for c in range(NCORES):
        r, k = c // CGRP, c % CGRP
        maps.append({
            "x_own": x[NLOC * r:NLOC * (r + 1)],
            "w_own": weight[WSH * k:WSH * (k + 1)],
            "bias2_rep": np.ascontiguousarray(np.broadcast_to(
                2.0 * bias[WSH * k:WSH * (k + 1)], (P, WSH))),
        })
    return maps


def kernel(x, weight, bias, _trace=False):
    nc = _get_nc()
    x = np.ascontiguousarray(np.asarray(x, dtype=np.float32))
    weight = np.ascontiguousarray(np.asarray(weight, dtype=np.float32))
    bias = np.asarray(bias, dtype=np.float32)

    res = run_bass_kernel_spmd(nc, _in_maps(x, weight, bias),
                               core_ids=list(range(NCORES)), trace=_trace)
    out = np.empty((N, OUT), dtype=np.float32)
    for c in range(NCORES):
        r, k = c // CGRP, c % CGRP
        out[NLOC * r:NLOC * (r + 1), WSH * k:WSH * (k + 1)] = \
            res.results[c]["out"]
    if _trace:
        return out, res
    return out


def _pjrt_runner(nc):
    """Return fn() that executes nc's NEFF once across the 8 cores."""
    import jax
    from jax.sharding import Mesh, PartitionSpec
    from jax.experimental.shard_map import shard_map
    from concourse import bass2jax, mybir as mb

    bass2jax.install_neuronx_cc_hook()
    partition_name = (nc.partition_id_tensor.name
                      if nc.partition_id_tensor else None)
    in_names, out_names, out_avals, zero_outs = [], [], [], []
    for alloc in nc.m.functions[0].allocations:
        if not isinstance(alloc, mb.MemoryLocationSet):
            continue
        name = alloc.memorylocations[0].name
        if alloc.kind == "ExternalInput":
            if name != partition_name:
                in_names.append(name)
        elif alloc.kind == "ExternalOutput":
            out_names.append(name)
            shape = tuple(alloc.tensor_shape)
            dtype = mb.dt.np(alloc.dtype)
            out_avals.append(jax.core.ShapedArray(shape, dtype))
            zero_outs.append(np.zeros(shape, dtype))
    n_params = len(in_names)
    all_names = tuple(in_names + out_names
                      + ([partition_name] if partition_name else []))

    def bodyfn(*args):
        extra = ([bass2jax.partition_id_tensor()] if partition_name else [])
        outs = bass2jax._bass_exec_p.bind(
            *args, *extra,
            out_avals=tuple(out_avals),
            in_names=all_names,
            out_names=tuple(out_names),
            lowering_input_output_aliases=(),
            sim_require_finite=True,
            sim_require_nnan=True,
            nc=nc,
        )
        return tuple(outs)

    devices = jax.devices()[:NCORES]
    mesh = Mesh(np.asarray(devices), ("core",))
    specs = (PartitionSpec("core"),) * (n_params + len(out_names))
    fn = jax.jit(shard_map(bodyfn, mesh=mesh, in_specs=specs,
                           out_specs=(PartitionSpec("core"),) * len(out_names),
                           check_rep=False), keep_unused=True)
    return fn, in_names, zero_outs


def _prep_exec(nc, x, weight, bias):
    import jax
    from jax.sharding import Mesh, PartitionSpec, NamedSharding

    fn, in_names, zero_outs = _pjrt_runner(nc)
    maps = _in_maps(x, weight, bias)
    concat_in = [np.concatenate([maps[c][n] for c in range(NCORES)], axis=0)
                 for n in in_names]
    concat_zeros = [np.zeros((NCORES * z.shape[0], *z.shape[1:]), z.dtype)
                    for z in zero_outs]
    mesh = Mesh(np.asarray(jax.devices()[:NCORES]), ("core",))
    sh = NamedSharding(mesh, PartitionSpec("core"))
    concat_in = [jax.device_put(a, sh) for a in concat_in]
    concat_zeros = [jax.device_put(a, sh) for a in concat_zeros]
    return fn, concat_in, concat_zeros


def time_kernel(x, weight, bias, reps_hi=1024, reps_lo=1, samples=6):
    """Per-execution device time via hardware-looped NEFFs: the same body
    runs reps_hi (resp. reps_lo) times inside one device program, so
    (wall_hi - wall_lo)/(reps_hi - reps_lo) cancels host dispatch cost.
    reps_hi is large enough that the device time dominates dispatch
    jitter by an order of magnitude."""
    import time
    import jax

    x = np.ascontiguousarray(np.asarray(x, dtype=np.float32))
    weight = np.ascontiguousarray(np.asarray(weight, dtype=np.float32))
    bias = np.asarray(bias, dtype=np.float32)

    runs = {}
    for k in (reps_lo, reps_hi):
        nc = build(loop_reps=k)
        runs[k] = _prep_exec(nc, x, weight, bias)
        out = runs[k][0](*runs[k][1], *runs[k][2])
        jax.block_until_ready(out)

    walls = {reps_lo: [], reps_hi: []}
    for _ in range(samples):
        for k in (reps_lo, reps_hi):
            fn, ci, cz = runs[k]
            t0 = time.perf_counter()
            out = fn(*ci, *cz)
            jax.block_until_ready(out)
            walls[k].append(time.perf_counter() - t0)

    diff = (min(walls[reps_hi]) - min(walls[reps_lo])) / (reps_hi - reps_lo)
    upper = min(walls[reps_hi]) / reps_hi
    per_exec = diff if diff > 0 else upper
    return per_exec, walls


# revision 16
# speedup vs baseline: 1.1087x; 1.1087x over previous
"""BFP-quantized linear (nn_BFPLinear) on 8 Trainium2 NeuronCores.

Math (must match reference exactly):
    xq = bfp_quant8_g64(x); wq = bfp_quant8_g64(weight)
    out = xq @ wq.T + 2*bias

Sharding (2 row-groups x 4 col-groups, core c = 4r+k), collective-free:
  - core c loads x rows [2048r, 2048(r+1)) and weight rows
    [1024k, 1024(k+1)) as fp32 and quantizes them on-chip (x is
    quantized redundantly by the 4 cores of each row group, w by the
    2 cores sharing a col shard; this removes all inter-core traffic).
  - output shard per core: out[2048r:2048(r+1), 1024k:1024(k+1)],
    written contiguously (no host-side permutation).

Quantization per group of 64 along `in` (bit-exact vs the fp32 ref):
    gmax = max|x|; p2 = exponent-only bits of gmax (= 2^e), clamped to
    >= 2^-126; inv = bitcast(0x7F000000 - p2) = 2^-e (exact);
    y = x * inv (exact power-of-2 scale);
    y = clip(y, -128.49/128, 127.49/128);
    m = (y + 1.5*2^16) - 1.5*2^16   (fused DVE round-to-nearest-even at
        ulp 2^-7, verified bit-exact on HW), cast to bf16 (exact);
    xq = m * bf16(p2) = round(x/step)*step with step = 2^(e-7), exact
        in bf16 (|m| <= 1, 8-bit mantissa, power-of-2 step).
The bf16 matmul inputs equal the reference's fp32 quantized values
bit-for-bit, so the only output error is fp32 summation order.

Transposes (quantized bf16 -> contraction-on-partition layout) are done
with SBUF->SBUF xbar DMA (dma_start_transpose), no DRAM staging.

build(loop_reps=N) wraps the whole body in a hardware For_i loop: one
NEFF executes the kernel N times back-to-back with no host round trip.
This is used by time_kernel() to measure per-execution device time far
below the host dispatch jitter. The graded kernel() path uses the plain
(no-loop) build.
"""
import sys

sys.path.insert(0, "/opt/trn_rl_repo")

import numpy as np

import concourse.bass as bass
import concourse.tile as tile
from concourse import mybir, bacc
from concourse.bass_utils import run_bass_kernel_spmd

# problem shape (hardcoded; kernel.py must be self-contained)
N = 4096
IN = 4096
OUT = 4096
NCORES = 8
RGRP = 2                    # row groups (x sharded 2-way)
CGRP = 4                    # col groups (weight sharded 4-way)
NLOC = N // RGRP            # 2048 output rows per core
WSH = OUT // CGRP           # 1024 output cols per core
J = 64                      # bfp group size
G = IN // J                 # 64 groups per row
KT = IN // 128              # 32 contraction tiles
P = 128
NXT = NLOC // P             # 16 x row-tiles per core
NWT = WSH // P              # 8 w row-tiles per core

MASK_EXP = 0x7F800000
MIN_NORM = 0x00800000
INV_C = 0x7F000000
MAGIC = float(np.float32(1.5 * 2.0 ** 16))
CLIP_HI = float(np.float32(127.49 / 128.0))
CLIP_LO = float(np.float32(-128.49 / 128.0))

_CACHE = {}


def build(loop_reps=None, ablate=None):
    """loop_reps=None: plain one-shot body (graded path).
    loop_reps=k: body wrapped in a hardware For_i executing k times.
    ablate: None | 'empty' | 'mmonly' | 'qtonly' | 'qonly' (perf probes)."""
    dt = mybir.dt
    nc = bacc.Bacc("TRN2", target_bir_lowering=False, debug=False)
    x_d = nc.dram_tensor("x_own", [NLOC, IN], dt.float32,
                         kind="ExternalInput").ap()
    w_d = nc.dram_tensor("w_own", [WSH, IN], dt.float32,
                         kind="ExternalInput").ap()
    b_d = nc.dram_tensor("bias2_rep", [P, WSH], dt.float32,
                         kind="ExternalInput").ap()
    out_d = nc.dram_tensor("out", [NLOC, WSH], dt.float32,
                           kind="ExternalOutput").ap()

    with tile.TileContext(nc) as tc:
        with tc.tile_pool(name="sb", bufs=1) as sb, \
             tc.tile_pool(name="inp", bufs=3) as inp, \
             tc.tile_pool(name="ypool", bufs=2) as ypool, \
             tc.tile_pool(name="mpool", bufs=2) as mpool, \
             tc.tile_pool(name="xqt", bufs=4) as xqtp, \
             tc.tile_pool(name="small", bufs=4) as small, \
             tc.tile_pool(name="otp", bufs=2) as otp, \
             tc.tile_pool(name="psum", bufs=4, space="PSUM") as psump:

            # constants (outside the timing loop)
            bias2 = sb.tile([P, WSH], dt.float32)
            nc.sync.dma_start(bias2[:], b_d)
            magic_p = sb.tile([P, 1], dt.float32)
            nc.vector.memset(magic_p[:], MAGIC)
            magic_n = sb.tile([P, 1], dt.float32)
            nc.vector.memset(magic_n[:], -MAGIC)
            # wqT[p, kt, o]: contraction i = kt*128+p, o = out col
            wqT = sb.tile([P, KT, WSH], dt.bfloat16)

            def quantize(src_d, row, ve):
                """Load fp32 rows [row*128,(row+1)*128) of src_d, return
                quantized bf16 tile [128, IN] (= m*step, bit-exact).
                ve: engine for the heavy elementwise ops (nc.vector or
                nc.gpsimd) -- whole tiles alternate between recipes so no
                tile has two writers (whole-tile dep tracking would
                serialize them)."""
                xt = inp.tile([P, IN], dt.float32, tag="in", name="in")
                nc.scalar.dma_start(xt[:], src_d[row * P:(row + 1) * P, :])
                x3 = xt[:].rearrange("p (g j) -> p g j", j=J)
                gmax = small.tile([P, G], dt.float32, tag="gmax", name="gmax")
                nc.vector.tensor_reduce(gmax[:], x3, mybir.AxisListType.X,
                                        mybir.AluOpType.max,
                                        apply_absolute_value=True)
                p2 = small.tile([P, G], dt.int32, tag="p2", name="p2")
                nc.vector.tensor_scalar(p2[:], gmax[:].bitcast(dt.int32),
                                        MASK_EXP, None,
                                        mybir.AluOpType.bitwise_and)
                nc.vector.tensor_scalar(p2[:], p2[:], MIN_NORM, None,
                                        mybir.AluOpType.max)
                inv_i = small.tile([P, G], dt.int32, tag="invi", name="invi")
                nc.vector.tensor_scalar(inv_i[:], p2[:], -1, INV_C,
                                        mybir.AluOpType.mult,
                                        mybir.AluOpType.add)
                step_bf = small.tile([P, G], dt.bfloat16, tag="stepbf",
                                     name="stepbf")
                nc.vector.tensor_copy(step_bf[:], p2[:].bitcast(dt.float32))

                yt = ypool.tile([P, IN], dt.float32, tag="y", name="y")
                y3 = yt[:].rearrange("p (g j) -> p g j", j=J)
                inv_b = (inv_i[:].bitcast(dt.float32).unsqueeze(2)
                         .broadcast_to([P, G, J]))
                ve.tensor_tensor(y3, x3, inv_b, mybir.AluOpType.mult)
                # round first (no pre-clip): |y| < 2 so m = rne(y*128)/128
                # stays in [-2, 2], exact in bf16; then saturate in bf16
                # -- equivalent to clip-before-round.
                nc.scalar.activation(yt[:], yt[:],
                                     mybir.ActivationFunctionType.Identity,
                                     bias=magic_p[:])
                mt = mpool.tile([P, IN], dt.bfloat16, tag="m", name="m")
                nc.scalar.activation(mt[:], yt[:],
                                     mybir.ActivationFunctionType.Identity,
                                     bias=magic_n[:])
                ve.tensor_scalar(mt[:], mt[:], 127.0 / 128.0, -1.0,
                                 mybir.AluOpType.min,
                                 mybir.AluOpType.max)
                m3 = mt[:].rearrange("p (g j) -> p g j", j=J)
                step_b = step_bf[:].unsqueeze(2).broadcast_to([P, G, J])
                ve.tensor_tensor(m3, m3, step_b, mybir.AluOpType.mult)
                return mt

            qcnt = [0]

            def _ve():
                qcnt[0] += 1
                return nc.vector if qcnt[0] % 2 else nc.gpsimd

            def do_w_tile(jw):
                wq = quantize(w_d, jw, _ve())
                nc.sync.dma_start_transpose(
                    wqT[:, :, jw * P:(jw + 1) * P], wq[:])

            def do_x_quant(jn):
                xq = quantize(x_d, jn, _ve())
                xqT = xqtp.tile([P, KT, P], dt.bfloat16, tag="xqT",
                                name="xqT")
                nc.sync.dma_start_transpose(xqT[:], xq[:])
                return xqT

            def do_x_mm(jn, xqT):
                ot = otp.tile([P, WSH], dt.float32, tag="ot", name="ot")
                for oh in range(2):
                    ps = psump.tile([P, 512], dt.float32, tag="ps", name="ps")
                    for kt in range(KT):
                        nc.tensor.matmul(
                            ps[:], xqT[:, kt, :],
                            wqT[:, kt, oh * 512:(oh + 1) * 512],
                            start=(kt == 0), stop=(kt == KT - 1))
                    nc.vector.tensor_tensor(
                        ot[:, oh * 512:(oh + 1) * 512], ps[:],
                        bias2[:, oh * 512:(oh + 1) * 512],
                        mybir.AluOpType.add)
                nc.scalar.dma_start(out_d[jn * P:(jn + 1) * P, :], ot[:])

            def body():
                # interleave w tiles with early x quantize/transpose so
                # all engines fill from the start. Matmuls are emitted
                # only after ALL w transposes: wqT is written in column
                # slices, and a matmul emitted earlier would read slices
                # not yet recorded as written (-> garbage).
                order = [("w", 0), ("w", 1), ("x", 0), ("w", 2), ("w", 3),
                         ("x", 1), ("w", 4), ("w", 5), ("x", 2),
                         ("w", 6), ("w", 7)]
                pending = {}
                for kind, idx in order:
                    if kind == "w":
                        do_w_tile(idx)
                    else:
                        pending[idx] = do_x_quant(idx)
                for jn in sorted(pending):
                    do_x_mm(jn, pending[jn])
                for jn in range(3, NXT):
                    xqT = do_x_quant(jn)
                    do_x_mm(jn, xqT)

            def body_empty():
                junk = otp.tile([P, 8], dt.float32, tag="ot", name="ot")
                nc.vector.memset(junk[:], 1.0)

            def body_mmonly(static_xqT):
                for jn in range(NXT):
                    do_x_mm(jn, static_xqT[jn % len(static_xqT)])

            def body_qt(with_transpose):
                for jw in range(NWT):
                    if with_transpose:
                        do_w_tile(jw)
                    else:
                        wq = quantize(w_d, jw, _ve())
                        nc.scalar.dma_start(
                            out_d[0:P, 64 * jw:64 * (jw + 1)],
                            wq[:, 0:128].bitcast(dt.float32))
                for jn in range(NXT):
                    if with_transpose:
                        xqT = do_x_quant(jn)
                        # tiny consumer so DCE keeps the chain
                        nc.scalar.dma_start(
                            out_d[jn * P:(jn + 1) * P, 0:64],
                            xqT[:, 0, :].bitcast(dt.float32))
                    else:
                        xq = quantize(x_d, jn, _ve())
                        nc.scalar.dma_start(
                            out_d[jn * P:(jn + 1) * P, 0:64],
                            xq[:, 0:128].bitcast(dt.float32))

            def body_op(op):
                xt = inp.tile([P, IN], dt.float32, tag="in", name="in")
                nc.scalar.dma_start(xt[:], x_d[0:P, :])
                x3 = xt[:].rearrange("p (g j) -> p g j", j=J)
                sc = small.tile([P, G], dt.float32, tag="gmax", name="gmax")
                nc.vector.memset(sc[:], 1.0)
                sc_b = sc[:].unsqueeze(2).broadcast_to([P, G, J])
                yt = ypool.tile([P, IN], dt.float32, tag="y", name="y")
                y3 = yt[:].rearrange("p (g j) -> p g j", j=J)
                mt = mpool.tile([P, IN], dt.bfloat16, tag="m", name="m")
                nc.vector.memset(mt[:], 0.5)
                m3 = mt[:].rearrange("p (g j) -> p g j", j=J)
                sbf = small.tile([P, G], dt.bfloat16, tag="stepbf",
                                 name="stepbf")
                nc.vector.memset(sbf[:], 1.0)
                sbf_b = sbf[:].unsqueeze(2).broadcast_to([P, G, J])
                for i in range(24):
                    if op == "gmult":
                        nc.gpsimd.tensor_tensor(y3, x3, sc_b,
                                                mybir.AluOpType.mult)
                    elif op == "dmult":
                        nc.vector.tensor_tensor(y3, x3, sc_b,
                                                mybir.AluOpType.mult)
                    elif op == "reduce":
                        nc.vector.tensor_reduce(sc[:], x3,
                                                mybir.AxisListType.X,
                                                mybir.AluOpType.max,
                                                apply_absolute_value=True)
                    elif op == "act":
                        nc.scalar.activation(
                            yt[:], xt[:],
                            mybir.ActivationFunctionType.Identity,
                            bias=magic_p[:])
                    elif op == "clip":
                        nc.vector.tensor_scalar(mt[:], mt[:], 127.0 / 128.0,
                                                -1.0, mybir.AluOpType.min,
                                                mybir.AluOpType.max)
                    elif op == "final":
                        nc.vector.tensor_tensor(m3, m3, sbf_b,
                                                mybir.AluOpType.mult)
                    elif op == "gfinal":
                        nc.gpsimd.tensor_tensor(m3, m3, sbf_b,
                                                mybir.AluOpType.mult)
                junk = otp.tile([P, 8], dt.float32, tag="ot", name="ot")
                nc.vector.tensor_copy(junk[:], yt[:, 0:8])
                nc.vector.tensor_copy(junk[:].bitcast(dt.bfloat16)[:, 0:8],
                                      mt[:, 0:8])
                nc.scalar.dma_start(out_d[0:P, 0:8], junk[:])

            if ablate is not None and ablate.startswith("op:"):
                opname = ablate[3:]
                chosen = lambda: body_op(opname)
            elif ablate == "mmonly":
                static_xqT = []
                for i in range(3):
                    t = xqtp.tile([P, KT, P], dt.bfloat16, tag="xqT",
                                  name="xqT")
                    nc.vector.memset(t[:].rearrange("p a b -> p (a b)"), 0.25)
                    static_xqT.append(t)
                nc.vector.memset(wqT[:].rearrange("p a b -> p (a b)"), 0.25)
                chosen = lambda: body_mmonly(static_xqT)
            elif ablate == "empty":
                chosen = body_empty
            elif ablate == "qtonly":
                chosen = lambda: body_qt(True)
            elif ablate == "qonly":
                chosen = lambda: body_qt(False)
            else:
                chosen = body

            if loop_reps is None:
                chosen()
            else:
                with tc.For_i(0, loop_reps, 1):
                    chosen()

    nc.compile()
    return nc


def _get_nc():
    if "nc" not in _CACHE:
        _CACHE["nc"] = build()
    return _CACHE["nc"]


def _in_maps(x, weight, bias):
    maps = []
    for c in range(NCORES):
        r, k = c // CGRP, c % CGRP
        maps.append({
            "x_own": x[NLOC * r:NLOC * (r + 1)],
            "w_own": weight[WSH * k:WSH * (k + 1)],
            "bias2_rep": np.ascontiguousarray(np.broadcast_to(
                2.0 * bias[WSH * k:WSH * (k + 1)], (P, WSH))),
        })
    return maps


def kernel(x, weight, bias, _trace=False):
    nc = _get_nc()
    x = np.ascontiguousarray(np.asarray(x, dtype=np.float32))
    weight = np.ascontiguousarray(np.asarray(weight, dtype=np.float32))
    bias = np.asarray(bias, dtype=np.float32)

    res = run_bass_kernel_spmd(nc, _in_maps(x, weight, bias),
                               core_ids=list(range(NCORES)), trace=_trace)
    out = np.empty((N, OUT), dtype=np.float32)
    for c in range(NCORES):
        r, k = c // CGRP, c % CGRP
        out[NLOC * r:NLOC * (r + 1), WSH * k:WSH * (k + 1)] = \
            res.results[c]["out"]
    if _trace:
        return out, res
    return out


def _pjrt_runner(nc):
    """Return fn() that executes nc's NEFF once across the 8 cores."""
    import jax
    from jax.sharding import Mesh, PartitionSpec
    from jax.experimental.shard_map import shard_map
    from concourse import bass2jax, mybir as mb

    bass2jax.install_neuronx_cc_hook()
    partition_name = (nc.partition_id_tensor.name
                      if nc.partition_id_tensor else None)
    in_names, out_names, out_avals, zero_outs = [], [], [], []
    for alloc in nc.m.functions[0].allocations:
        if not isinstance(alloc, mb.MemoryLocationSet):
            continue
        name = alloc.memorylocations[0].name
        if alloc.kind == "ExternalInput":
            if name != partition_name:
                in_names.append(name)
        elif alloc.kind == "ExternalOutput":
            out_names.append(name)
            shape = tuple(alloc.tensor_shape)
            dtype = mb.dt.np(alloc.dtype)
            out_avals.append(jax.core.ShapedArray(shape, dtype))
            zero_outs.append(np.zeros(shape, dtype))
    n_params = len(in_names)
    all_names = tuple(in_names + out_names
                      + ([partition_name] if partition_name else []))

    def bodyfn(*args):
        extra = ([bass2jax.partition_id_tensor()] if partition_name else [])
        outs = bass2jax._bass_exec_p.bind(
            *args, *extra,
            out_avals=tuple(out_avals),
            in_names=all_names,
            out_names=tuple(out_names),
            lowering_input_output_aliases=(),
            sim_require_finite=True,
            sim_require_nnan=True,
            nc=nc,
        )
        return tuple(outs)

    devices = jax.devices()[:NCORES]
    mesh = Mesh(np.asarray(devices), ("core",))
    specs = (PartitionSpec("core"),) * (n_params + len(out_names))
    fn = jax.jit(shard_map(bodyfn, mesh=mesh, in_specs=specs,
                           out_specs=(PartitionSpec("core"),) * len(out_names),
                           check_rep=False), keep_unused=True)
    return fn, in_names, zero_outs


def _prep_exec(nc, x, weight, bias):
    import jax
    from jax.sharding import Mesh, PartitionSpec, NamedSharding

    fn, in_names, zero_outs = _pjrt_runner(nc)
    maps = _in_maps(x, weight, bias)
    concat_in = [np.concatenate([maps[c][n] for c in range(NCORES)], axis=0)
                 for n in in_names]
    concat_zeros = [np.zeros((NCORES * z.shape[0], *z.shape[1:]), z.dtype)
                    for z in zero_outs]
    mesh = Mesh(np.asarray(jax.devices()[:NCORES]), ("core",))
    sh = NamedSharding(mesh, PartitionSpec("core"))
    concat_in = [jax.device_put(a, sh) for a in concat_in]
    concat_zeros = [jax.device_put(a, sh) for a in concat_zeros]
    return fn, concat_in, concat_zeros


def time_kernel(x, weight, bias, reps_hi=1024, reps_lo=1, samples=6):
    """Per-execution device time via hardware-looped NEFFs: the same body
    runs reps_hi (resp. reps_lo) times inside one device program, so
    (wall_hi - wall_lo)/(reps_hi - reps_lo) cancels host dispatch cost.
    reps_hi is large enough that the device time dominates dispatch
    jitter by an order of magnitude."""
    import time
    import jax

    x = np.ascontiguousarray(np.asarray(x, dtype=np.float32))
    weight = np.ascontiguousarray(np.asarray(weight, dtype=np.float32))
    bias = np.asarray(bias, dtype=np.float32)

    runs = {}
    for k in (reps_lo, reps_hi):
        nc = build(loop_reps=k)
        runs[k] = _prep_exec(nc, x, weight, bias)
        out = runs[k][0](*runs[k][1], *runs[k][2])
        jax.block_until_ready(out)

    walls = {reps_lo: [], reps_hi: []}
    for _ in range(samples):
        for k in (reps_lo, reps_hi):
            fn, ci, cz = runs[k]
            t0 = time.perf_counter()
            out = fn(*ci, *cz)
            jax.block_until_ready(out)
            walls[k].append(time.perf_counter() - t0)

    diff = (min(walls[reps_hi]) - min(walls[reps_lo])) / (reps_hi - reps_lo)
    upper = min(walls[reps_hi]) / reps_hi
    per_exec = diff if diff > 0 else upper
    return per_exec, walls


# revision 17
# speedup vs baseline: 1.2095x; 1.0910x over previous
"""BFP-quantized linear (nn_BFPLinear) on 8 Trainium2 NeuronCores.

Math (must match reference exactly):
    xq = bfp_quant8_g64(x); wq = bfp_quant8_g64(weight)
    out = xq @ wq.T + 2*bias

Sharding (2 row-groups x 4 col-groups, core c = 4r+k), collective-free:
  - core c loads x rows [2048r, 2048(r+1)) and weight rows
    [1024k, 1024(k+1)) as fp32 and quantizes them on-chip (x is
    quantized redundantly by the 4 cores of each row group, w by the
    2 cores sharing a col shard; this removes all inter-core traffic).
  - output shard per core: out[2048r:2048(r+1), 1024k:1024(k+1)],
    written contiguously (no host-side permutation).

Quantization per group of 64 along `in` (bit-exact vs the fp32 ref):
    gmax = max|x|; p2 = exponent-only bits of gmax (= 2^e), clamped to
    >= 2^-126; inv = bitcast(0x7F000000 - p2) = 2^-e (exact);
    y = x * inv (exact power-of-2 scale);
    y = clip(y, -128.49/128, 127.49/128);
    m = (y + 1.5*2^16) - 1.5*2^16   (fused DVE round-to-nearest-even at
        ulp 2^-7, verified bit-exact on HW), cast to bf16 (exact);
    xq = m * bf16(p2) = round(x/step)*step with step = 2^(e-7), exact
        in bf16 (|m| <= 1, 8-bit mantissa, power-of-2 step).
The bf16 matmul inputs equal the reference's fp32 quantized values
bit-for-bit, so the only output error is fp32 summation order.

Transposes (quantized bf16 -> contraction-on-partition layout) are done
with SBUF->SBUF xbar DMA (dma_start_transpose), no DRAM staging.

build(loop_reps=N) wraps the whole body in a hardware For_i loop: one
NEFF executes the kernel N times back-to-back with no host round trip.
This is used by time_kernel() to measure per-execution device time far
below the host dispatch jitter. The graded kernel() path uses the plain
(no-loop) build.
"""
import sys

sys.path.insert(0, "/opt/trn_rl_repo")

import numpy as np

import concourse.bass as bass
import concourse.tile as tile
from concourse import mybir, bacc
from concourse.bass_utils import run_bass_kernel_spmd

# problem shape (hardcoded; kernel.py must be self-contained)
N = 4096
IN = 4096
OUT = 4096
NCORES = 8
RGRP = 2                    # row groups (x sharded 2-way)
CGRP = 4                    # col groups (weight sharded 4-way)
NLOC = N // RGRP            # 2048 output rows per core
WSH = OUT // CGRP           # 1024 output cols per core
J = 64                      # bfp group size
G = IN // J                 # 64 groups per row
KT = IN // 128              # 32 contraction tiles
P = 128
NXT = NLOC // P             # 16 x row-tiles per core
NWT = WSH // P              # 8 w row-tiles per core

MASK_EXP = 0x7F800000
MIN_NORM = 0x00800000
INV_C = 0x7F000000
MAGIC = float(np.float32(1.5 * 2.0 ** 16))
CLIP_HI = float(np.float32(127.49 / 128.0))
CLIP_LO = float(np.float32(-128.49 / 128.0))

_CACHE = {}


def build(loop_reps=None, ablate=None):
    """loop_reps=None: plain one-shot body (graded path).
    loop_reps=k: body wrapped in a hardware For_i executing k times.
    ablate: None | 'empty' | 'mmonly' | 'qtonly' | 'qonly' (perf probes)."""
    dt = mybir.dt
    nc = bacc.Bacc("TRN2", target_bir_lowering=False, debug=False)
    x_d = nc.dram_tensor("x_own", [NLOC, IN], dt.float32,
                         kind="ExternalInput").ap()
    w_d = nc.dram_tensor("w_own", [WSH, IN], dt.float32,
                         kind="ExternalInput").ap()
    b_d = nc.dram_tensor("bias2_rep", [P, WSH], dt.float32,
                         kind="ExternalInput").ap()
    out_d = nc.dram_tensor("out", [NLOC, WSH], dt.float32,
                           kind="ExternalOutput").ap()

    with tile.TileContext(nc) as tc:
        with tc.tile_pool(name="sb", bufs=1) as sb, \
             tc.tile_pool(name="inp", bufs=3) as inp, \
             tc.tile_pool(name="ypool", bufs=2) as ypool, \
             tc.tile_pool(name="mpool", bufs=2) as mpool, \
             tc.tile_pool(name="xqt", bufs=4) as xqtp, \
             tc.tile_pool(name="small", bufs=4) as small, \
             tc.tile_pool(name="otp", bufs=2) as otp, \
             tc.tile_pool(name="psum", bufs=4, space="PSUM") as psump:

            # constants (outside the timing loop)
            bias2 = sb.tile([P, WSH], dt.float32)
            nc.sync.dma_start(bias2[:], b_d)
            magic_p = sb.tile([P, 1], dt.float32)
            nc.vector.memset(magic_p[:], MAGIC)
            magic_n = sb.tile([P, 1], dt.float32)
            nc.vector.memset(magic_n[:], -MAGIC)
            # wqT[p, kt, o]: contraction i = kt*128+p, o = out col
            wqT = sb.tile([P, KT, WSH], dt.bfloat16)

            def quantize(src_d, row):
                """Load fp32 rows [row*128,(row+1)*128) of src_d, return
                quantized bf16 tile [128, IN] (= m*step, bit-exact)."""
                xt = inp.tile([P, IN], dt.float32, tag="in", name="in")
                nc.scalar.dma_start(xt[:], src_d[row * P:(row + 1) * P, :])
                x3 = xt[:].rearrange("p (g j) -> p g j", j=J)
                gmax = small.tile([P, G], dt.float32, tag="gmax", name="gmax")
                nc.vector.tensor_reduce(gmax[:], x3, mybir.AxisListType.X,
                                        mybir.AluOpType.max,
                                        apply_absolute_value=True)
                p2 = small.tile([P, G], dt.int32, tag="p2", name="p2")
                nc.vector.tensor_scalar(p2[:], gmax[:].bitcast(dt.int32),
                                        MASK_EXP, None,
                                        mybir.AluOpType.bitwise_and)
                nc.vector.tensor_scalar(p2[:], p2[:], MIN_NORM, None,
                                        mybir.AluOpType.max)
                inv_i = small.tile([P, G], dt.int32, tag="invi", name="invi")
                nc.vector.tensor_scalar(inv_i[:], p2[:], -1, INV_C,
                                        mybir.AluOpType.mult,
                                        mybir.AluOpType.add)
                step_bf = small.tile([P, G], dt.bfloat16, tag="stepbf",
                                     name="stepbf")
                nc.vector.tensor_copy(step_bf[:], p2[:].bitcast(dt.float32))

                yt = ypool.tile([P, IN], dt.float32, tag="y", name="y")
                y3 = yt[:].rearrange("p (g j) -> p g j", j=J)
                inv_b = (inv_i[:].bitcast(dt.float32).unsqueeze(2)
                         .broadcast_to([P, G, J]))
                nc.gpsimd.tensor_tensor(y3, x3, inv_b, mybir.AluOpType.mult)
                # round first (no pre-clip): |y| < 2 so m = rne(y*128)/128
                # stays in [-2, 2], exact in bf16; then saturate in bf16
                # -- equivalent to clip-before-round.
                nc.scalar.activation(yt[:], yt[:],
                                     mybir.ActivationFunctionType.Identity,
                                     bias=magic_p[:])
                mt = mpool.tile([P, IN], dt.bfloat16, tag="m", name="m")
                nc.scalar.activation(mt[:], yt[:],
                                     mybir.ActivationFunctionType.Identity,
                                     bias=magic_n[:])
                nc.vector.tensor_scalar(mt[:], mt[:], 127.0 / 128.0, -1.0,
                                        mybir.AluOpType.min,
                                        mybir.AluOpType.max)
                m3 = mt[:].rearrange("p (g j) -> p g j", j=J)
                step_b = step_bf[:].unsqueeze(2).broadcast_to([P, G, J])
                # final m *= step split DVE/GpSimd to balance engine load
                GS = 40
                nc.vector.tensor_tensor(m3[:, 0:GS, :], m3[:, 0:GS, :],
                                        step_b[:, 0:GS, :],
                                        mybir.AluOpType.mult)
                nc.gpsimd.tensor_tensor(m3[:, GS:G, :], m3[:, GS:G, :],
                                        step_b[:, GS:G, :],
                                        mybir.AluOpType.mult)
                return mt

            def do_w_tile(jw):
                wq = quantize(w_d, jw)
                nc.sync.dma_start_transpose(
                    wqT[:, :, jw * P:(jw + 1) * P], wq[:])

            def do_x_quant(jn):
                xq = quantize(x_d, jn)
                xqT = xqtp.tile([P, KT, P], dt.bfloat16, tag="xqT",
                                name="xqT")
                nc.sync.dma_start_transpose(xqT[:], xq[:])
                return xqT

            def do_x_mm(jn, xqT):
                ot = otp.tile([P, WSH], dt.float32, tag="ot", name="ot")
                for oh in range(2):
                    ps = psump.tile([P, 512], dt.float32, tag="ps", name="ps")
                    for kt in range(KT):
                        nc.tensor.matmul(
                            ps[:], xqT[:, kt, :],
                            wqT[:, kt, oh * 512:(oh + 1) * 512],
                            start=(kt == 0), stop=(kt == KT - 1))
                    nc.vector.tensor_tensor(
                        ot[:, oh * 512:(oh + 1) * 512], ps[:],
                        bias2[:, oh * 512:(oh + 1) * 512],
                        mybir.AluOpType.add)
                nc.scalar.dma_start(out_d[jn * P:(jn + 1) * P, :], ot[:])

            def body():
                # interleave w tiles with early x quantize/transpose so
                # all engines fill from the start. Matmuls are emitted
                # only after ALL w transposes: wqT is written in column
                # slices, and a matmul emitted earlier would read slices
                # not yet recorded as written (-> garbage).
                order = [("w", 0), ("w", 1), ("x", 0), ("w", 2), ("w", 3),
                         ("x", 1), ("w", 4), ("w", 5), ("x", 2),
                         ("w", 6), ("w", 7)]
                pending = {}
                for kind, idx in order:
                    if kind == "w":
                        do_w_tile(idx)
                    else:
                        pending[idx] = do_x_quant(idx)
                for jn in sorted(pending):
                    do_x_mm(jn, pending[jn])
                for jn in range(3, NXT):
                    xqT = do_x_quant(jn)
                    do_x_mm(jn, xqT)

            def body_empty():
                junk = otp.tile([P, 8], dt.float32, tag="ot", name="ot")
                nc.vector.memset(junk[:], 1.0)

            def body_mmonly(static_xqT):
                for jn in range(NXT):
                    do_x_mm(jn, static_xqT[jn % len(static_xqT)])

            def body_qt(with_transpose):
                for jw in range(NWT):
                    if with_transpose:
                        do_w_tile(jw)
                    else:
                        wq = quantize(w_d, jw)
                        nc.scalar.dma_start(
                            out_d[0:P, 64 * jw:64 * (jw + 1)],
                            wq[:, 0:128].bitcast(dt.float32))
                for jn in range(NXT):
                    if with_transpose:
                        xqT = do_x_quant(jn)
                        # tiny consumer so DCE keeps the chain
                        nc.scalar.dma_start(
                            out_d[jn * P:(jn + 1) * P, 0:64],
                            xqT[:, 0, :].bitcast(dt.float32))
                    else:
                        xq = quantize(x_d, jn)
                        nc.scalar.dma_start(
                            out_d[jn * P:(jn + 1) * P, 0:64],
                            xq[:, 0:128].bitcast(dt.float32))

            def body_op(op):
                xt = inp.tile([P, IN], dt.float32, tag="in", name="in")
                nc.scalar.dma_start(xt[:], x_d[0:P, :])
                x3 = xt[:].rearrange("p (g j) -> p g j", j=J)
                sc = small.tile([P, G], dt.float32, tag="gmax", name="gmax")
                nc.vector.memset(sc[:], 1.0)
                sc_b = sc[:].unsqueeze(2).broadcast_to([P, G, J])
                yt = ypool.tile([P, IN], dt.float32, tag="y", name="y")
                y3 = yt[:].rearrange("p (g j) -> p g j", j=J)
                mt = mpool.tile([P, IN], dt.bfloat16, tag="m", name="m")
                nc.vector.memset(mt[:], 0.5)
                m3 = mt[:].rearrange("p (g j) -> p g j", j=J)
                sbf = small.tile([P, G], dt.bfloat16, tag="stepbf",
                                 name="stepbf")
                nc.vector.memset(sbf[:], 1.0)
                sbf_b = sbf[:].unsqueeze(2).broadcast_to([P, G, J])
                for i in range(24):
                    if op == "gmult":
                        nc.gpsimd.tensor_tensor(y3, x3, sc_b,
                                                mybir.AluOpType.mult)
                    elif op == "dmult":
                        nc.vector.tensor_tensor(y3, x3, sc_b,
                                                mybir.AluOpType.mult)
                    elif op == "reduce":
                        nc.vector.tensor_reduce(sc[:], x3,
                                                mybir.AxisListType.X,
                                                mybir.AluOpType.max,
                                                apply_absolute_value=True)
                    elif op == "act":
                        nc.scalar.activation(
                            yt[:], xt[:],
                            mybir.ActivationFunctionType.Identity,
                            bias=magic_p[:])
                    elif op == "clip":
                        nc.vector.tensor_scalar(mt[:], mt[:], 127.0 / 128.0,
                                                -1.0, mybir.AluOpType.min,
                                                mybir.AluOpType.max)
                    elif op == "final":
                        nc.vector.tensor_tensor(m3, m3, sbf_b,
                                                mybir.AluOpType.mult)
                    elif op == "gfinal":
                        nc.gpsimd.tensor_tensor(m3, m3, sbf_b,
                                                mybir.AluOpType.mult)
                junk = otp.tile([P, 8], dt.float32, tag="ot", name="ot")
                nc.vector.tensor_copy(junk[:], yt[:, 0:8])
                nc.vector.tensor_copy(junk[:].bitcast(dt.bfloat16)[:, 0:8],
                                      mt[:, 0:8])
                nc.scalar.dma_start(out_d[0:P, 0:8], junk[:])

            if ablate is not None and ablate.startswith("op:"):
                opname = ablate[3:]
                chosen = lambda: body_op(opname)
            elif ablate == "mmonly":
                static_xqT = []
                for i in range(3):
                    t = xqtp.tile([P, KT, P], dt.bfloat16, tag="xqT",
                                  name="xqT")
                    nc.vector.memset(t[:].rearrange("p a b -> p (a b)"), 0.25)
                    static_xqT.append(t)
                nc.vector.memset(wqT[:].rearrange("p a b -> p (a b)"), 0.25)
                chosen = lambda: body_mmonly(static_xqT)
            elif ablate == "empty":
                chosen = body_empty
            elif ablate == "qtonly":
                chosen = lambda: body_qt(True)
            elif ablate == "qonly":
                chosen = lambda: body_qt(False)
            else:
                chosen = body

            if loop_reps is None:
                chosen()
            else:
                with tc.For_i(0, loop_reps, 1):
                    chosen()

    nc.compile()
    return nc


def _get_nc():
    if "nc" not in _CACHE:
        _CACHE["nc"] = build()
    return _CACHE["nc"]


def _in_maps(x, weight, bias):
    maps = []
    for c in range(NCORES):
        r, k = c // CGRP, c % CGRP
        maps.append({
            "x_own": x[NLOC * r:NLOC * (r + 1)],
            "w_own": weight[WSH * k:WSH * (k + 1)],
            "bias2_rep": np.ascontiguousarray(np.broadcast_to(
                2.0 * bias[WSH * k:WSH * (k + 1)], (P, WSH))),
        })
    return maps


def kernel(x, weight, bias, _trace=False):
    nc = _get_nc()
    x = np.ascontiguousarray(np.asarray(x, dtype=np.float32))
    weight = np.ascontiguousarray(np.asarray(weight, dtype=np.float32))
    bias = np.asarray(bias, dtype=np.float32)

    res = run_bass_kernel_spmd(nc, _in_maps(x, weight, bias),
                               core_ids=list(range(NCORES)), trace=_trace)
    out = np.empty((N, OUT), dtype=np.float32)
    for c in range(NCORES):
        r, k = c // CGRP, c % CGRP
        out[NLOC * r:NLOC * (r + 1), WSH * k:WSH * (k + 1)] = \
            res.results[c]["out"]
    if _trace:
        return out, res
    return out


def _pjrt_runner(nc):
    """Return fn() that executes nc's NEFF once across the 8 cores."""
    import jax
    from jax.sharding import Mesh, PartitionSpec
    from jax.experimental.shard_map import shard_map
    from concourse import bass2jax, mybir as mb

    bass2jax.install_neuronx_cc_hook()
    partition_name = (nc.partition_id_tensor.name
                      if nc.partition_id_tensor else None)
    in_names, out_names, out_avals, zero_outs = [], [], [], []
    for alloc in nc.m.functions[0].allocations:
        if not isinstance(alloc, mb.MemoryLocationSet):
            continue
        name = alloc.memorylocations[0].name
        if alloc.kind == "ExternalInput":
            if name != partition_name:
                in_names.append(name)
        elif alloc.kind == "ExternalOutput":
            out_names.append(name)
            shape = tuple(alloc.tensor_shape)
            dtype = mb.dt.np(alloc.dtype)
            out_avals.append(jax.core.ShapedArray(shape, dtype))
            zero_outs.append(np.zeros(shape, dtype))
    n_params = len(in_names)
    all_names = tuple(in_names + out_names
                      + ([partition_name] if partition_name else []))

    def bodyfn(*args):
        extra = ([bass2jax.partition_id_tensor()] if partition_name else [])
        outs = bass2jax._bass_exec_p.bind(
            *args, *extra,
            out_avals=tuple(out_avals),
            in_names=all_names,
            out_names=tuple(out_names),
            lowering_input_output_aliases=(),
            sim_require_finite=True,
            sim_require_nnan=True,
            nc=nc,
        )
        return tuple(outs)

    devices = jax.devices()[:NCORES]
    mesh = Mesh(np.asarray(devices), ("core",))
    specs = (PartitionSpec("core"),) * (n_params + len(out_names))
    fn = jax.jit(shard_map(bodyfn, mesh=mesh, in_specs=specs,
                           out_specs=(PartitionSpec("core"),) * len(out_names),
                           check_rep=False), keep_unused=True)
    return fn, in_names, zero_outs


def _prep_exec(nc, x, weight, bias):
    import jax
    from jax.sharding import Mesh, PartitionSpec, NamedSharding

    fn, in_names, zero_outs = _pjrt_runner(nc)
    maps = _in_maps(x, weight, bias)
    concat_in = [np.concatenate([maps[c][n] for c in range(NCORES)], axis=0)
                 for n in in_names]
    concat_zeros = [np.zeros((NCORES * z.shape[0], *z.shape[1:]), z.dtype)
                    for z in zero_outs]
    mesh = Mesh(np.asarray(jax.devices()[:NCORES]), ("core",))
    sh = NamedSharding(mesh, PartitionSpec("core"))
    concat_in = [jax.device_put(a, sh) for a in concat_in]
    concat_zeros = [jax.device_put(a, sh) for a in concat_zeros]
    return fn, concat_in, concat_zeros


def time_kernel(x, weight, bias, reps_hi=1024, reps_lo=1, samples=6):
    """Per-execution device time via hardware-looped NEFFs: the same body
    runs reps_hi (resp. reps_lo) times inside one device program, so
    (wall_hi - wall_lo)/(reps_hi - reps_lo) cancels host dispatch cost.
    reps_hi is large enough that the device time dominates dispatch
    jitter by an order of magnitude."""
    import time
    import jax

    x = np.ascontiguousarray(np.asarray(x, dtype=np.float32))
    weight = np.ascontiguousarray(np.asarray(weight, dtype=np.float32))
    bias = np.asarray(bias, dtype=np.float32)

    runs = {}
    for k in (reps_lo, reps_hi):
        nc = build(loop_reps=k)
        runs[k] = _prep_exec(nc, x, weight, bias)
        out = runs[k][0](*runs[k][1], *runs[k][2])
        jax.block_until_ready(out)

    walls = {reps_lo: [], reps_hi: []}
    for _ in range(samples):
        for k in (reps_lo, reps_hi):
            fn, ci, cz = runs[k]
            t0 = time.perf_counter()
            out = fn(*ci, *cz)
            jax.block_until_ready(out)
            walls[k].append(time.perf_counter() - t0)

    diff = (min(walls[reps_hi]) - min(walls[reps_lo])) / (reps_hi - reps_lo)
    upper = min(walls[reps_hi]) / reps_hi
    per_exec = diff if diff > 0 else upper
    return per_exec, walls
